# revision 18
# baseline (speedup 1.0000x reference)
"""Trainium2 Bass kernel for nn_CSPCompBlock (dense depthwise-conv CSP block).

Strategy (pure data parallelism, one batch image per NeuronCore):
  - Each of the 3 "pseudo" stages = 4 depthwise 3-tap convs + BN + residual ReLU.
  - Depthwise conv as TensorEngine matmuls: lhsT = diagonal weight matrix
    (per-channel tap weight on the diagonal), rhs = shifted image tile
    (shift = free-dim AP offset), 3 taps accumulate in one PSUM group.
    Residual added via an extra identity matmul into the same PSUM group.
    BN folded into conv4's diagonal weights + bias.
  - fp32r matmul dtype: full PE rate with ~2^-11 rounding (measured ~5e-4
    end-to-end absmax error vs fp32 reference). fp32r matmul dst must be a
    contiguous PSUM range: column taps read zero-halo cols (tile width 132)
    instead of clipping, row taps clip at image edges (dst stays contiguous).
  - Stage-to-stage "transposes" ([C,(H,W)] -> [W,(C,H)] -> [H,(C,W)]) run
    on-chip: PE transpose-mode 128x128 tiles into PSUM (4 packed per bank),
    copied back to a full-image SBUF tile. Output block2 is exactly stage3's
    input layout, so it is DMA'd straight out with 512B-contiguous
    descriptors.
  - Epilogues (relu(psum+bias)) batched over 1024-col (2-bank) PSUM chunks,
    alternating ScalarE/VectorE.
"""
import sys

for _p in ("/opt/trn_rl_repo", "/opt/pypackages"):
    if _p not in sys.path:
        sys.path.insert(0, _p)

import numpy as np

import concourse.bacc as bacc
import concourse.bass as bass
import concourse.mybir as mybir
import concourse.tile as tile
from concourse.bass_utils import run_bass_kernel_spmd

F32 = mybir.dt.float32
F32R = mybir.dt.float32r
RELU = mybir.ActivationFunctionType.Relu
ADD = mybir.AluOpType.add
MAX = mybir.AluOpType.max

P = 128          # partitions = per-stage channel dim (C, W, H resp.)
S = 128          # spatial extent (all dims are 128)
HALO = 2         # zero cols each side of work tiles
SW = S + 2 * HALO
GROW = 4         # matmul-group rows (4*128 = 512 = one PSUM bank)
CHUNK = 8        # epilogue chunk rows (8*128 = 1024 = two PSUM banks)
N_MAT = 37       # 3 stages * 4 convs * 3 taps + identity
IDENT = 36
EPS = 1e-3


class _Epi:
    """Round-robin ACT/DVE epilogue + copy dispatcher."""

    def __init__(self, nc):
        self.nc = nc
        self.i = 0

    def relu_bias(self, out_ap, psum_ap, bias_ap):
        # out = relu(psum + bias)
        self.i += 1
        if self.i % 2 == 0:
            self.nc.scalar.activation(out_ap, psum_ap, RELU, bias=bias_ap, scale=1.0)
        else:
            self.nc.vector.tensor_scalar(out_ap, psum_ap, bias_ap, 0.0,
                                         op0=ADD, op1=MAX)

    def copy(self, out_ap, psum_ap):
        self.i += 1
        if self.i % 2 == 0:
            self.nc.scalar.copy(out_ap, psum_ap)
        else:
            self.nc.vector.tensor_copy(out_ap, psum_ap)


def _conv_band(nc, epi, psum_pool, diags, biases, mat_base, bias_idx,
               in_tile, in_base, in_lo, in_hi, in_halo,
               out_tile, out_base, out_lo, out_hi, out_halo,
               axis, dil, resid=None):
    """One depthwise conv over a band of rows, via diag matmuls.

    Matmul groups of <=GROW rows (one PSUM bank each); two groups share a
    [P, CHUNK*S] psum chunk; one epilogue per chunk.
    axis 'row': taps shift across tile rows (clipped at [in_lo, in_hi)).
    axis 'col': taps shift within rows into the zero halo (in_halo required).
    resid: (tile, base, halo) rows added via identity matmul.
    """
    ioff = HALO if in_halo else 0
    ooff = HALO if out_halo else 0
    taps = ((1, 0), (0, -dil), (2, dil))  # center first: carries start=True
    if axis == 'col':
        assert in_halo

    chunks = []   # (c0, c1, pstile)
    groups = []   # (g0, g1, pstile, psoff)
    c0 = out_lo
    while c0 < out_hi:
        c1 = min(c0 + CHUNK, out_hi)
        ps = psum_pool.tile([P, CHUNK * S], F32, tag="psum", name="psg")
        chunks.append((c0, c1, ps))
        g0 = c0
        while g0 < c1:
            g1 = min(g0 + GROW, c1)
            groups.append((g0, g1, ps, (g0 - c0) * S))
            g0 = g1
        c0 = c1

    emissions = [[] for _ in groups]
    for ti, (t, off) in enumerate(taps):
        lhsT = diags[:, (mat_base + t) * P:(mat_base + t + 1) * P]
        for gi, (g0, g1, ps, po) in enumerate(groups):
            if axis == 'row':
                s0 = max(g0 + off, in_lo)
                s1 = min(g1 + off, in_hi)
                if s1 <= s0:
                    continue
                rhs = in_tile[:, s0 - in_base:s1 - in_base, ioff:ioff + S]
                out = ps[:, po + (s0 - off - g0) * S:po + (s1 - off - g0) * S]
            else:
                rhs = in_tile[:, g0 - in_base:g1 - in_base,
                              ioff + off:ioff + off + S]
                out = ps[:, po:po + (g1 - g0) * S]
            emissions[gi].append((ti, lhsT, out, rhs))
    if resid is not None:
        ident = diags[:, IDENT * P:(IDENT + 1) * P]
        r_tile, r_base, r_halo = resid
        roff = HALO if r_halo else 0
        for gi, (g0, g1, ps, po) in enumerate(groups):
            rhs = r_tile[:, g0 - r_base:g1 - r_base, roff:roff + S]
            emissions[gi].append((3, ident, ps[:, po:po + (g1 - g0) * S], rhs))

    # Emit tap-major (weight-load batching); start/stop flags per group.
    flat = []
    for gi, ems in enumerate(emissions):
        assert ems and ems[0][0] == 0, "center tap must exist"
        for j, (ti, lhsT, out, rhs) in enumerate(ems):
            flat.append((ti, gi, lhsT, out, rhs, j == 0, j == len(ems) - 1))
    flat.sort(key=lambda e: (e[0], e[1]))
    for ti, gi, lhsT, out, rhs, is_start, is_stop in flat:
        nc.tensor.matmul(out, lhsT, rhs, start=is_start, stop=is_stop)

    bias_ap = biases[:, bias_idx:bias_idx + 1]
    for c0, c1, ps in chunks:
        epi.relu_bias(
            out_tile[:, c0 - out_base:c1 - out_base, ooff:ooff + S],
            ps[:, 0:(c1 - c0) * S], bias_ap)


def _stage(nc, tc, epi, psum_pool, psum_t, diags, biases, identr, stage,
           pw, band, in_full, x_in, out3, handoff_full):
    """Emit one pseudo stage. stage: 0, 1, or 2."""
    mb = stage * 12
    bb = stage * 4
    nband = S // band

    def mk(rows, width, tag, n):
        ts = [pw.tile([P, rows, width], F32R, tag=f"{tag}{i}_{stage}",
                      name=tag) for i in range(n)]
        for t in ts:
            if width == SW:
                nc.vector.memset(t[:, :, 0:HALO].bitcast(F32), 0.0)
                nc.vector.memset(t[:, :, HALO + S:SW].bitcast(F32), 0.0)
        return ts

    # double-buffer where the SBUF budget allows (stage 0 has more room:
    # the full3 pool is not yet open)
    dbl = stage == 0
    c1s = mk(band + 4, SW, "c1", 2)
    c2s = mk(band + 4, SW, "c2", 2 if dbl else 1)
    c3s = mk(band, SW, "c3", 2 if dbl else 1)
    oos = [pw.tile([P, band, S], F32R, tag=f"oo{i}_{stage}", name="oo")
           for i in range(2)]
    if stage == 0:
        xts = mk(band + 6, SW, "xt", 2)
        for t in xts:
            nc.vector.memset(t[:].bitcast(F32), 0.0)

    for bi in range(nband):
        r0 = bi * band
        pp = bi % 2
        c1 = c1s[pp % len(c1s)]
        c2 = c2s[pp % len(c2s)]
        c3 = c3s[pp % len(c3s)]
        oo = oos[pp]
        if stage == 0:
            in_lo, in_hi = max(0, r0 - 3), min(S, r0 + band + 3)
            in_base = r0 - 3
            xt = xts[pp]
            nc.sync.dma_start(
                xt[:, in_lo - in_base:in_hi - in_base, HALO:HALO + S],
                x_in[:, in_lo:in_hi, :].bitcast(F32R))
            src, s_base, s_lo, s_hi, s_halo = xt, in_base, in_lo, in_hi, True
        else:
            src, s_base, s_lo, s_hi, s_halo = in_full, 0, 0, S, False

        c_lo, c_hi = max(0, r0 - 2), min(S, r0 + band + 2)
        c_base = r0 - 2
        _conv_band(nc, epi, psum_pool, diags, biases, mb + 0, bb + 0,
                   src, s_base, s_lo, s_hi, s_halo,
                   c1, c_base, c_lo, c_hi, True, 'row', 1)
        _conv_band(nc, epi, psum_pool, diags, biases, mb + 3, bb + 1,
                   c1, c_base, c_lo, c_hi, True,
                   c2, c_base, c_lo, c_hi, True, 'col', 1)
        _conv_band(nc, epi, psum_pool, diags, biases, mb + 6, bb + 2,
                   c2, c_base, c_lo, c_hi, True,
                   c3, r0, r0, r0 + band, True, 'row', 2)
        _conv_band(nc, epi, psum_pool, diags, biases, mb + 9, bb + 3,
                   c3, r0, r0, r0 + band, True,
                   oo, r0, r0, r0 + band, False, 'col', 2,
                   resid=(src, s_base, s_halo))

        if stage == 0:
            nc.sync.dma_start(out3[0:P, r0:r0 + band, :], oo[:].bitcast(F32))
        elif stage == 2:
            nc.sync.dma_start(
                out3[2 * P + r0:2 * P + r0 + band, :, :]
                .rearrange("c h w -> h c w"),
                oo[:].bitcast(F32))
        if stage != 2:
            # handoff: per row-slice PE transpose, 4 packed per PSUM bank
            for q0 in range(0, band, 4):
                pt = psum_t.tile([P, 4 * P], F32R, tag="pt", name="pt")
                for q in range(4):
                    nc.tensor.transpose(pt[:, q * P:(q + 1) * P],
                                        oo[:, q0 + q, :], identr[:])
                if stage == 0:
                    # in2_full[w, c, h]: dst iterates (c, h-in-pack);
                    # pack holds (q=h-slice, c) -> permute source
                    epi.copy(handoff_full[:, :, r0 + q0:r0 + q0 + 4],
                             pt[:].rearrange("p (q c) -> p c q", c=P))
                else:
                    # in3_full[h, c, w]: dst iterates (c-in-pack, w);
                    # pack holds (q=c-slice, w) -> orders already match
                    epi.copy(handoff_full[:, r0 + q0:r0 + q0 + 4, :], pt[:])


def _build_nc():
    nc = bacc.Bacc(None, target_bir_lowering=False)
    x_in = nc.dram_tensor("x", [P, S, S], F32, kind="ExternalInput")
    diags_in = nc.dram_tensor("diags", [P, N_MAT * P], F32, kind="ExternalInput")
    biases_in = nc.dram_tensor("biases", [P, 12], F32, kind="ExternalInput")
    out3 = nc.dram_tensor("out3", [3 * P, S, S], F32, kind="ExternalOutput")

    with tile.TileContext(nc) as tc:
        with tc.tile_pool(name="consts", bufs=1) as cpool, \
             tc.tile_pool(name="full2", bufs=1) as p_in2, \
             tc.tile_pool(name="psum", bufs=3, space="PSUM") as psum_pool, \
             tc.tile_pool(name="psum_t", bufs=2, space="PSUM") as psum_t:

            diags = cpool.tile([P, N_MAT * P], F32R, tag="diags")
            biases = cpool.tile([P, 12], F32, tag="biases")
            identr = cpool.tile([P, P], F32R, tag="identr")
            nc.sync.dma_start(diags[:], diags_in[:].bitcast(F32R))
            nc.sync.dma_start(biases[:], biases_in[:])
            nc.sync.dma_start(identr[:],
                              diags_in[:, IDENT * P:(IDENT + 1) * P].bitcast(F32R))

            in2_full = p_in2.tile([P, S, S], F32R, tag="in2")   # [w, c, h]
            epi = _Epi(nc)

            # Phase 1: stage 1 (partitions=C, rows=H, cols=W)
            with tc.tile_pool(name="s1work", bufs=1) as pw:
                _stage(nc, tc, epi, psum_pool, psum_t, diags, biases, identr,
                       0, pw, 16, None, x_in, out3, in2_full)

            # full3 pool opens only now: pools reserve SBUF from their open
            # point, and phase 1 needs the space for 32-row bands.
            with tc.tile_pool(name="full3", bufs=1) as p_in3:
                # Phase 2: stage 2 (partitions=W, rows=C, cols=H), 16-row bands
                in3_full = p_in3.tile([P, S, S], F32R, tag="in3")   # [h, c, w]
                with tc.tile_pool(name="s2work", bufs=1) as pw:
                    _stage(nc, tc, epi, psum_pool, psum_t, diags, biases,
                           identr, 1, pw, 16, in2_full, None, out3, in3_full)

                # block2 = in3_full[h, c, w] -> out3[P + c, h, w]
                nc.sync.dma_start(
                    out3[P:2 * P, :, :].rearrange("c h w -> h c w"),
                    in3_full[:].bitcast(F32))

                # Phase 3: stage 3 (partitions=H, rows=C, cols=W), 16-row bands
                with tc.tile_pool(name="s3work", bufs=1) as pw:
                    _stage(nc, tc, epi, psum_pool, psum_t, diags, biases,
                           identr, 2, pw, 16, in3_full, None, out3, None)
    nc.finalize()
    return nc


def _prep_weights(inputs):
    """Build diag matrices [P, N_MAT*P] and biases [P, 12] (BN folded)."""
    diags = np.zeros((P, N_MAT * P), np.float32)
    biases = np.zeros((P, 12), np.float32)
    idx = np.arange(P)
    for s, pfx in enumerate(("hw", "ch", "cw")):
        ws = [np.asarray(inputs[f"{pfx}_w{j}"]).reshape(P, 3) for j in (1, 2, 3, 4)]
        bs = [np.asarray(inputs[f"{pfx}_b{j}"]).astype(np.float32) for j in (1, 2, 3, 4)]
        g = np.asarray(inputs[f"{pfx}_g"])
        bt = np.asarray(inputs[f"{pfx}_bt"])
        m = np.asarray(inputs[f"{pfx}_m"])
        v = np.asarray(inputs[f"{pfx}_v"])
        scale = (g / np.sqrt(v + EPS)).astype(np.float32)
        for k in range(4):
            w = ws[k].astype(np.float32)
            if k == 3:
                w = w * scale[:, None]
                biases[:, s * 4 + k] = (bs[3] - m) * scale + bt
            else:
                biases[:, s * 4 + k] = bs[k]
            for t in range(3):
                j = s * 12 + k * 3 + t
                diags[idx, j * P + idx] = w[:, t]
    diags[idx, IDENT * P + idx] = 1.0
    return diags, biases


_NC_CACHE = None
TRACE = False            # set True (e.g. from test.py) to capture an NTFF profile
LAST_RESULT = None       # BassKernelResults of the most recent kernel() call


def _get_nc():
    global _NC_CACHE
    if _NC_CACHE is None:
        _NC_CACHE = _build_nc()
    return _NC_CACHE


def kernel(**inputs):
    global LAST_RESULT
    x = np.asarray(inputs["x"], np.float32)          # [8, 128, 128, 128]
    B = x.shape[0]
    diags, biases = _prep_weights(inputs)
    nc = _get_nc()
    in_maps = [{"x": np.ascontiguousarray(x[b]), "diags": diags,
                "biases": biases} for b in range(B)]
    res = run_bass_kernel_spmd(nc, in_maps, core_ids=list(range(B)),
                               trace=TRACE)
    LAST_RESULT = res
    out = np.empty((B, 4 * P, S, S), np.float32)
    out[:, :P] = x
    for b in range(B):
        out[b, P:] = res.results[b]["out3"]
    return out


# revision 19
# speedup vs baseline: 1.1464x; 1.1464x over previous
"""Trainium2 Bass kernel for nn_CSPCompBlock (dense depthwise-conv CSP block).

Strategy (pure data parallelism, one batch image per NeuronCore):
  - Each of the 3 "pseudo" stages = 4 depthwise 3-tap convs + BN + residual ReLU.
  - Depthwise conv as TensorEngine matmuls: lhsT = diagonal weight matrix
    (per-channel tap weight on the diagonal), rhs = shifted image tile
    (shift = free-dim AP offset), 3 taps accumulate in one PSUM group.
    Residual added via an extra identity matmul into the same PSUM group.
    BN folded into conv4's diagonal weights + bias.
  - fp32r matmul dtype: full PE rate with ~2^-11 rounding (measured ~5e-4
    end-to-end absmax error vs fp32 reference). fp32r matmul dst must be a
    contiguous PSUM range: column taps read zero-halo cols (tile width 132)
    instead of clipping, row taps clip at image edges (dst stays contiguous).
  - Stage-to-stage "transposes" ([C,(H,W)] -> [W,(C,H)] -> [H,(C,W)]) run
    on-chip: PE transpose-mode 128x128 tiles into PSUM (4 packed per bank),
    copied back to a full-image SBUF tile. Output block2 is exactly stage3's
    input layout, so it is DMA'd straight out with 512B-contiguous
    descriptors.
  - Epilogues (relu(psum+bias)) batched over 1024-col (2-bank) PSUM chunks,
    alternating ScalarE/VectorE.
"""
import sys

for _p in ("/opt/trn_rl_repo", "/opt/pypackages"):
    if _p not in sys.path:
        sys.path.insert(0, _p)

import numpy as np

import concourse.bacc as bacc
import concourse.bass as bass
import concourse.mybir as mybir
import concourse.tile as tile
from concourse.bass_utils import run_bass_kernel_spmd

F32 = mybir.dt.float32
F32R = mybir.dt.float32r
RELU = mybir.ActivationFunctionType.Relu
ADD = mybir.AluOpType.add
MAX = mybir.AluOpType.max

P = 128          # partitions = per-stage channel dim (C, W, H resp.)
S = 128          # spatial extent (all dims are 128)
HALO = 2         # zero cols each side of work tiles
SW = S + 2 * HALO
GROW = 4         # matmul-group rows (4*128 = 512 = one PSUM bank)
CHUNK = 4        # epilogue chunk rows (4*128 = 512 = one PSUM bank)
N_MAT = 37       # 3 stages * 4 convs * 3 taps + identity
IDENT = 36
EPS = 1e-3


class _Epi:
    """Round-robin ACT/DVE epilogue + copy dispatcher."""

    def __init__(self, nc):
        self.nc = nc
        self.i = 0

    def relu_bias(self, out_ap, psum_ap, bias_ap):
        # out = relu(psum + bias)
        self.i += 1
        if self.i % 2 == 0:
            self.nc.scalar.activation(out_ap, psum_ap, RELU, bias=bias_ap, scale=1.0)
        else:
            self.nc.vector.tensor_scalar(out_ap, psum_ap, bias_ap, 0.0,
                                         op0=ADD, op1=MAX)

    def copy(self, out_ap, psum_ap):
        self.i += 1
        if self.i % 2 == 0:
            self.nc.scalar.copy(out_ap, psum_ap)
        else:
            self.nc.vector.tensor_copy(out_ap, psum_ap)


def _conv_band(nc, epi, psum_pool, diags, biases, mat_base, bias_idx,
               in_tile, in_base, in_lo, in_hi, in_halo,
               out_tile, out_base, out_lo, out_hi, out_halo,
               axis, dil, resid=None):
    """One depthwise conv over a band of rows, via diag matmuls.

    Matmul groups of <=GROW rows (one PSUM bank each); two groups share a
    [P, CHUNK*S] psum chunk; one epilogue per chunk.
    axis 'row': taps shift across tile rows (clipped at [in_lo, in_hi)).
    axis 'col': taps shift within rows into the zero halo (in_halo required).
    resid: (tile, base, halo) rows added via identity matmul.
    """
    ioff = HALO if in_halo else 0
    ooff = HALO if out_halo else 0
    taps = ((1, 0), (0, -dil), (2, dil))  # center first: carries start=True
    if axis == 'col':
        assert in_halo

    chunks = []   # (c0, c1, pstile)
    groups = []   # (g0, g1, pstile, psoff)
    c0 = out_lo
    while c0 < out_hi:
        c1 = min(c0 + CHUNK, out_hi)
        ps = psum_pool.tile([P, CHUNK * S], F32, tag="psum", name="psg")
        chunks.append((c0, c1, ps))
        g0 = c0
        while g0 < c1:
            g1 = min(g0 + GROW, c1)
            groups.append((g0, g1, ps, (g0 - c0) * S))
            g0 = g1
        c0 = c1

    emissions = [[] for _ in groups]
    for ti, (t, off) in enumerate(taps):
        lhsT = diags[:, (mat_base + t) * P:(mat_base + t + 1) * P]
        for gi, (g0, g1, ps, po) in enumerate(groups):
            if axis == 'row':
                s0 = max(g0 + off, in_lo)
                s1 = min(g1 + off, in_hi)
                if s1 <= s0:
                    continue
                rhs = in_tile[:, s0 - in_base:s1 - in_base, ioff:ioff + S]
                out = ps[:, po + (s0 - off - g0) * S:po + (s1 - off - g0) * S]
            else:
                rhs = in_tile[:, g0 - in_base:g1 - in_base,
                              ioff + off:ioff + off + S]
                out = ps[:, po:po + (g1 - g0) * S]
            emissions[gi].append((ti, lhsT, out, rhs))
    if resid is not None:
        ident = diags[:, IDENT * P:(IDENT + 1) * P]
        r_tile, r_base, r_halo = resid
        roff = HALO if r_halo else 0
        for gi, (g0, g1, ps, po) in enumerate(groups):
            rhs = r_tile[:, g0 - r_base:g1 - r_base, roff:roff + S]
            emissions[gi].append((3, ident, ps[:, po:po + (g1 - g0) * S], rhs))

    # Emit tap-major (weight-load batching); start/stop flags per group.
    flat = []
    for gi, ems in enumerate(emissions):
        assert ems and ems[0][0] == 0, "center tap must exist"
        for j, (ti, lhsT, out, rhs) in enumerate(ems):
            flat.append((ti, gi, lhsT, out, rhs, j == 0, j == len(ems) - 1))
    flat.sort(key=lambda e: (e[0], e[1]))
    for ti, gi, lhsT, out, rhs, is_start, is_stop in flat:
        nc.tensor.matmul(out, lhsT, rhs, start=is_start, stop=is_stop)

    bias_ap = biases[:, bias_idx:bias_idx + 1]
    for c0, c1, ps in chunks:
        epi.relu_bias(
            out_tile[:, c0 - out_base:c1 - out_base, ooff:ooff + S],
            ps[:, 0:(c1 - c0) * S], bias_ap)


def _stage(nc, tc, epi, psum_pool, psum_t, diags, biases, identr, stage,
           pw, band, in_full, x_in, out3, handoff_full):
    """Emit one pseudo stage. stage: 0, 1, or 2."""
    mb = stage * 12
    bb = stage * 4
    nband = S // band

    def mk(rows, width, tag, n):
        ts = [pw.tile([P, rows, width], F32R, tag=f"{tag}{i}_{stage}",
                      name=tag) for i in range(n)]
        for t in ts:
            if width == SW:
                nc.vector.memset(t[:, :, 0:HALO].bitcast(F32), 0.0)
                nc.vector.memset(t[:, :, HALO + S:SW].bitcast(F32), 0.0)
        return ts

    # double-buffer where the SBUF budget allows (stage 0 has more room:
    # the full3 pool is not yet open)
    dbl = stage == 0
    c1s = mk(band + 4, SW, "c1", 2)
    c2s = mk(band + 4, SW, "c2", 2 if dbl else 1)
    c3s = mk(band, SW, "c3", 2 if dbl else 1)
    oos = [pw.tile([P, band, S], F32R, tag=f"oo{i}_{stage}", name="oo")
           for i in range(2)]
    if stage == 0:
        xts = mk(band + 6, SW, "xt", 2)
        for t in xts:
            nc.vector.memset(t[:].bitcast(F32), 0.0)

    for bi in range(nband):
        r0 = bi * band
        pp = bi % 2
        c1 = c1s[pp % len(c1s)]
        c2 = c2s[pp % len(c2s)]
        c3 = c3s[pp % len(c3s)]
        oo = oos[pp]
        if stage == 0:
            in_lo, in_hi = max(0, r0 - 3), min(S, r0 + band + 3)
            in_base = r0 - 3
            xt = xts[pp]
            nc.sync.dma_start(
                xt[:, in_lo - in_base:in_hi - in_base, HALO:HALO + S],
                x_in[:, in_lo:in_hi, :].bitcast(F32R))
            src, s_base, s_lo, s_hi, s_halo = xt, in_base, in_lo, in_hi, True
        else:
            src, s_base, s_lo, s_hi, s_halo = in_full, 0, 0, S, False

        c_lo, c_hi = max(0, r0 - 2), min(S, r0 + band + 2)
        c_base = r0 - 2
        _conv_band(nc, epi, psum_pool, diags, biases, mb + 0, bb + 0,
                   src, s_base, s_lo, s_hi, s_halo,
                   c1, c_base, c_lo, c_hi, True, 'row', 1)
        _conv_band(nc, epi, psum_pool, diags, biases, mb + 3, bb + 1,
                   c1, c_base, c_lo, c_hi, True,
                   c2, c_base, c_lo, c_hi, True, 'col', 1)
        _conv_band(nc, epi, psum_pool, diags, biases, mb + 6, bb + 2,
                   c2, c_base, c_lo, c_hi, True,
                   c3, r0, r0, r0 + band, True, 'row', 2)
        _conv_band(nc, epi, psum_pool, diags, biases, mb + 9, bb + 3,
                   c3, r0, r0, r0 + band, True,
                   oo, r0, r0, r0 + band, False, 'col', 2,
                   resid=(src, s_base, s_halo))

        if stage == 0:
            nc.sync.dma_start(out3[0:P, r0:r0 + band, :], oo[:].bitcast(F32))
        elif stage == 2:
            nc.sync.dma_start(
                out3[2 * P + r0:2 * P + r0 + band, :, :]
                .rearrange("c h w -> h c w"),
                oo[:].bitcast(F32))
        if stage != 2:
            # handoff: per row-slice PE transpose, 4 packed per PSUM bank
            for q0 in range(0, band, 4):
                pt = psum_t.tile([P, 4 * P], F32R, tag="pt", name="pt")
                for q in range(4):
                    nc.tensor.transpose(pt[:, q * P:(q + 1) * P],
                                        oo[:, q0 + q, :], identr[:])
                if stage == 0:
                    # in2_full[w, c, h]: dst iterates (c, h-in-pack);
                    # pack holds (q=h-slice, c) -> permute source
                    epi.copy(handoff_full[:, :, r0 + q0:r0 + q0 + 4],
                             pt[:].rearrange("p (q c) -> p c q", c=P))
                else:
                    # in3_full[h, c, w]: dst iterates (c-in-pack, w);
                    # pack holds (q=c-slice, w) -> orders already match
                    epi.copy(handoff_full[:, r0 + q0:r0 + q0 + 4, :], pt[:])


def _build_nc():
    nc = bacc.Bacc(None, target_bir_lowering=False)
    x_in = nc.dram_tensor("x", [P, S, S], F32, kind="ExternalInput")
    diags_in = nc.dram_tensor("diags", [P, N_MAT * P], F32, kind="ExternalInput")
    biases_in = nc.dram_tensor("biases", [P, 12], F32, kind="ExternalInput")
    out3 = nc.dram_tensor("out3", [3 * P, S, S], F32, kind="ExternalOutput")

    with tile.TileContext(nc) as tc:
        with tc.tile_pool(name="consts", bufs=1) as cpool, \
             tc.tile_pool(name="full2", bufs=1) as p_in2, \
             tc.tile_pool(name="psum", bufs=6, space="PSUM") as psum_pool, \
             tc.tile_pool(name="psum_t", bufs=2, space="PSUM") as psum_t:

            diags = cpool.tile([P, N_MAT * P], F32R, tag="diags")
            biases = cpool.tile([P, 12], F32, tag="biases")
            identr = cpool.tile([P, P], F32R, tag="identr")
            nc.sync.dma_start(diags[:], diags_in[:].bitcast(F32R))
            nc.sync.dma_start(biases[:], biases_in[:])
            nc.sync.dma_start(identr[:],
                              diags_in[:, IDENT * P:(IDENT + 1) * P].bitcast(F32R))

            in2_full = p_in2.tile([P, S, S], F32R, tag="in2")   # [w, c, h]
            epi = _Epi(nc)

            # Phase 1: stage 1 (partitions=C, rows=H, cols=W)
            with tc.tile_pool(name="s1work", bufs=1) as pw:
                _stage(nc, tc, epi, psum_pool, psum_t, diags, biases, identr,
                       0, pw, 16, None, x_in, out3, in2_full)

            # full3 pool opens only now: pools reserve SBUF from their open
            # point, and phase 1 needs the space for 32-row bands.
            with tc.tile_pool(name="full3", bufs=1) as p_in3:
                # Phase 2: stage 2 (partitions=W, rows=C, cols=H), 16-row bands
                in3_full = p_in3.tile([P, S, S], F32R, tag="in3")   # [h, c, w]
                with tc.tile_pool(name="s2work", bufs=1) as pw:
                    _stage(nc, tc, epi, psum_pool, psum_t, diags, biases,
                           identr, 1, pw, 16, in2_full, None, out3, in3_full)

                # block2 = in3_full[h, c, w] -> out3[P + c, h, w]
                nc.sync.dma_start(
                    out3[P:2 * P, :, :].rearrange("c h w -> h c w"),
                    in3_full[:].bitcast(F32))

                # Phase 3: stage 3 (partitions=H, rows=C, cols=W), 16-row bands
                with tc.tile_pool(name="s3work", bufs=1) as pw:
                    _stage(nc, tc, epi, psum_pool, psum_t, diags, biases,
                           identr, 2, pw, 16, in3_full, None, out3, None)
    nc.finalize()
    return nc


def _prep_weights(inputs):
    """Build diag matrices [P, N_MAT*P] and biases [P, 12] (BN folded)."""
    diags = np.zeros((P, N_MAT * P), np.float32)
    biases = np.zeros((P, 12), np.float32)
    idx = np.arange(P)
    for s, pfx in enumerate(("hw", "ch", "cw")):
        ws = [np.asarray(inputs[f"{pfx}_w{j}"]).reshape(P, 3) for j in (1, 2, 3, 4)]
        bs = [np.asarray(inputs[f"{pfx}_b{j}"]).astype(np.float32) for j in (1, 2, 3, 4)]
        g = np.asarray(inputs[f"{pfx}_g"])
        bt = np.asarray(inputs[f"{pfx}_bt"])
        m = np.asarray(inputs[f"{pfx}_m"])
        v = np.asarray(inputs[f"{pfx}_v"])
        scale = (g / np.sqrt(v + EPS)).astype(np.float32)
        for k in range(4):
            w = ws[k].astype(np.float32)
            if k == 3:
                w = w * scale[:, None]
                biases[:, s * 4 + k] = (bs[3] - m) * scale + bt
            else:
                biases[:, s * 4 + k] = bs[k]
            for t in range(3):
                j = s * 12 + k * 3 + t
                diags[idx, j * P + idx] = w[:, t]
    diags[idx, IDENT * P + idx] = 1.0
    return diags, biases


_NC_CACHE = None
TRACE = False            # set True (e.g. from test.py) to capture an NTFF profile
LAST_RESULT = None       # BassKernelResults of the most recent kernel() call


def _get_nc():
    global _NC_CACHE
    if _NC_CACHE is None:
        _NC_CACHE = _build_nc()
    return _NC_CACHE


def kernel(**inputs):
    global LAST_RESULT
    x = np.asarray(inputs["x"], np.float32)          # [8, 128, 128, 128]
    B = x.shape[0]
    diags, biases = _prep_weights(inputs)
    nc = _get_nc()
    in_maps = [{"x": np.ascontiguousarray(x[b]), "diags": diags,
                "biases": biases} for b in range(B)]
    res = run_bass_kernel_spmd(nc, in_maps, core_ids=list(range(B)),
                               trace=TRACE)
    LAST_RESULT = res
    out = np.empty((B, 4 * P, S, S), np.float32)
    out[:, :P] = x
    for b in range(B):
        out[b, P:] = res.results[b]["out3"]
    return out


# revision 21
# speedup vs baseline: 1.2244x; 1.0681x over previous
"""Trainium2 Bass kernel for nn_CSPCompBlock (dense depthwise-conv CSP block).

Strategy (pure data parallelism, one batch image per NeuronCore):
  - Each of the 3 "pseudo" stages = 4 depthwise 3-tap convs + BN + residual ReLU.
  - Depthwise conv as TensorEngine matmuls: lhsT = diagonal weight matrix
    (per-channel tap weight on the diagonal), rhs = shifted image tile
    (shift = free-dim AP offset), 3 taps accumulate in one PSUM group.
    Residual added via an extra identity matmul into the same PSUM group.
    BN folded into conv4's diagonal weights + bias.
  - fp32r matmul dtype: full PE rate with ~2^-11 rounding (measured ~5e-4
    end-to-end absmax error vs fp32 reference). fp32r matmul dst must be a
    contiguous PSUM range: column taps read zero-halo cols (tile width 132)
    instead of clipping, row taps clip at image edges (dst stays contiguous).
  - Stage-to-stage "transposes" ([C,(H,W)] -> [W,(C,H)] -> [H,(C,W)]) run
    on-chip: PE transpose-mode 128x128 tiles into PSUM (4 packed per bank),
    copied back to a full-image SBUF tile. Output block2 is exactly stage3's
    input layout, so it is DMA'd straight out with 512B-contiguous
    descriptors.
  - Epilogues (relu(psum+bias)) batched over 1024-col (2-bank) PSUM chunks,
    alternating ScalarE/VectorE.
"""
import sys

for _p in ("/opt/trn_rl_repo", "/opt/pypackages"):
    if _p not in sys.path:
        sys.path.insert(0, _p)

import numpy as np

import concourse.bacc as bacc
import concourse.bass as bass
import concourse.mybir as mybir
import concourse.tile as tile
from concourse.bass_utils import run_bass_kernel_spmd

F32 = mybir.dt.float32
F32R = mybir.dt.float32r
RELU = mybir.ActivationFunctionType.Relu
ADD = mybir.AluOpType.add
MAX = mybir.AluOpType.max

P = 128          # partitions = per-stage channel dim (C, W, H resp.)
S = 128          # spatial extent (all dims are 128)
HALO = 2         # zero cols each side of work tiles
SW = S + 2 * HALO
GROW = 4         # matmul-group rows (4*128 = 512 = one PSUM bank)
CHUNK = 4        # epilogue chunk rows (4*128 = 512 = one PSUM bank)
N_MAT = 37       # 3 stages * 4 convs * 3 taps + identity
IDENT = 36
EPS = 1e-3


class _Epi:
    """Round-robin ACT/DVE epilogue + copy dispatcher."""

    def __init__(self, nc):
        self.nc = nc
        self.i = 0

    def relu_bias(self, out_ap, psum_ap, bias_ap):
        # out = relu(psum + bias); bias_ap may be 0.0 (already folded)
        self.i += 1
        if self.i % 5 < 3:   # weight toward ACT: DVE also runs residual STTs
            self.nc.scalar.activation(out_ap, psum_ap, RELU, bias=bias_ap, scale=1.0)
        elif isinstance(bias_ap, float):
            self.nc.vector.tensor_scalar(out_ap, psum_ap, bias_ap, None, op0=MAX)
        else:
            self.nc.vector.tensor_scalar(out_ap, psum_ap, bias_ap, 0.0,
                                         op0=ADD, op1=MAX)

    def copy(self, out_ap, psum_ap):
        self.i += 1
        if self.i % 2 == 0:
            self.nc.scalar.copy(out_ap, psum_ap)
        else:
            self.nc.vector.tensor_copy(out_ap, psum_ap)


def _conv_band(nc, epi, psum_pool, diags, biases, mat_base, bias_idx,
               in_tile, in_base, in_lo, in_hi, in_halo,
               out_tile, out_base, out_lo, out_hi, out_halo,
               axis, dil, resid=None):
    """One depthwise conv over a band of rows, via diag matmuls.

    Matmul groups of <=GROW rows (one PSUM bank each); two groups share a
    [P, CHUNK*S] psum chunk; one epilogue per chunk.
    axis 'row': taps shift across tile rows (clipped at [in_lo, in_hi)).
    axis 'col': taps shift within rows into the zero halo (in_halo required).
    resid: (tile, base, halo) rows added via identity matmul.
    """
    ioff = HALO if in_halo else 0
    ooff = HALO if out_halo else 0
    taps = ((1, 0), (0, -dil), (2, dil))  # center first: carries start=True
    if axis == 'col':
        assert in_halo

    chunks = []   # (c0, c1, pstile)
    groups = []   # (g0, g1, pstile, psoff)
    c0 = out_lo
    while c0 < out_hi:
        c1 = min(c0 + CHUNK, out_hi)
        ps = psum_pool.tile([P, CHUNK * S], F32, tag="psum", name="psg")
        chunks.append((c0, c1, ps))
        g0 = c0
        while g0 < c1:
            g1 = min(g0 + GROW, c1)
            groups.append((g0, g1, ps, (g0 - c0) * S))
            g0 = g1
        c0 = c1

    emissions = [[] for _ in groups]
    for ti, (t, off) in enumerate(taps):
        lhsT = diags[:, (mat_base + t) * P:(mat_base + t + 1) * P]
        for gi, (g0, g1, ps, po) in enumerate(groups):
            if axis == 'row':
                s0 = max(g0 + off, in_lo)
                s1 = min(g1 + off, in_hi)
                if s1 <= s0:
                    continue
                rhs = in_tile[:, s0 - in_base:s1 - in_base, ioff:ioff + S]
                out = ps[:, po + (s0 - off - g0) * S:po + (s1 - off - g0) * S]
            else:
                rhs = in_tile[:, g0 - in_base:g1 - in_base,
                              ioff + off:ioff + off + S]
                out = ps[:, po:po + (g1 - g0) * S]
            emissions[gi].append((ti, lhsT, out, rhs))
    # Emit tap-major (weight-load batching); start/stop flags per group.
    flat = []
    for gi, ems in enumerate(emissions):
        assert ems and ems[0][0] == 0, "center tap must exist"
        for j, (ti, lhsT, out, rhs) in enumerate(ems):
            flat.append((ti, gi, lhsT, out, rhs, j == 0, j == len(ems) - 1))
    flat.sort(key=lambda e: (e[0], e[1]))
    for ti, gi, lhsT, out, rhs, is_start, is_stop in flat:
        nc.tensor.matmul(out, lhsT, rhs, start=is_start, stop=is_stop)

    bias_ap = biases[:, bias_idx:bias_idx + 1]
    for c0, c1, ps in chunks:
        psv = ps[:, 0:(c1 - c0) * S]
        if resid is not None:
            # psum += bias + residual rows, fused on DVE; epilogue is
            # then a plain ReLU
            r_tile, r_base, r_halo = resid
            roff = HALO if r_halo else 0
            rhs = r_tile[:, c0 - r_base:c1 - r_base, roff:roff + S]
            nc.vector.scalar_tensor_tensor(
                psv, psv, bias_ap, rhs.bitcast(F32), op0=ADD, op1=ADD)
            epi.relu_bias(
                out_tile[:, c0 - out_base:c1 - out_base, ooff:ooff + S],
                psv, 0.0)
        else:
            epi.relu_bias(
                out_tile[:, c0 - out_base:c1 - out_base, ooff:ooff + S],
                psv, bias_ap)


def _stage(nc, tc, epi, psum_pool, psum_t, diags, biases, identr, stage,
           pw, band, in_full, x_in, out3, handoff_full):
    """Emit one pseudo stage. stage: 0, 1, or 2."""
    mb = stage * 12
    bb = stage * 4
    nband = S // band

    def mk(rows, width, tag, n):
        ts = [pw.tile([P, rows, width], F32R, tag=f"{tag}{i}_{stage}",
                      name=tag) for i in range(n)]
        for t in ts:
            if width == SW:
                nc.vector.memset(t[:, :, 0:HALO].bitcast(F32), 0.0)
                nc.vector.memset(t[:, :, HALO + S:SW].bitcast(F32), 0.0)
        return ts

    # double-buffer where the SBUF budget allows (stage 0 has more room:
    # the full3 pool is not yet open)
    dbl = stage == 0
    c1s = mk(band + 4, SW, "c1", 2)
    c2s = mk(band + 4, SW, "c2", 2 if dbl else 1)
    c3s = mk(band, SW, "c3", 2 if dbl else 1)
    oos = [pw.tile([P, band, S], F32R, tag=f"oo{i}_{stage}", name="oo")
           for i in range(2)]
    if stage == 0:
        xts = mk(band + 6, SW, "xt", 2)
        for t in xts:
            nc.vector.memset(t[:].bitcast(F32), 0.0)

    for bi in range(nband):
        r0 = bi * band
        pp = bi % 2
        c1 = c1s[pp % len(c1s)]
        c2 = c2s[pp % len(c2s)]
        c3 = c3s[pp % len(c3s)]
        oo = oos[pp]
        if stage == 0:
            in_lo, in_hi = max(0, r0 - 3), min(S, r0 + band + 3)
            in_base = r0 - 3
            xt = xts[pp]
            nc.sync.dma_start(
                xt[:, in_lo - in_base:in_hi - in_base, HALO:HALO + S],
                x_in[:, in_lo:in_hi, :].bitcast(F32R))
            src, s_base, s_lo, s_hi, s_halo = xt, in_base, in_lo, in_hi, True
        else:
            src, s_base, s_lo, s_hi, s_halo = in_full, 0, 0, S, False

        c_lo, c_hi = max(0, r0 - 2), min(S, r0 + band + 2)
        c_base = r0 - 2
        _conv_band(nc, epi, psum_pool, diags, biases, mb + 0, bb + 0,
                   src, s_base, s_lo, s_hi, s_halo,
                   c1, c_base, c_lo, c_hi, True, 'row', 1)
        _conv_band(nc, epi, psum_pool, diags, biases, mb + 3, bb + 1,
                   c1, c_base, c_lo, c_hi, True,
                   c2, c_base, c_lo, c_hi, True, 'col', 1)
        _conv_band(nc, epi, psum_pool, diags, biases, mb + 6, bb + 2,
                   c2, c_base, c_lo, c_hi, True,
                   c3, r0, r0, r0 + band, True, 'row', 2)
        _conv_band(nc, epi, psum_pool, diags, biases, mb + 9, bb + 3,
                   c3, r0, r0, r0 + band, True,
                   oo, r0, r0, r0 + band, False, 'col', 2,
                   resid=(src, s_base, s_halo))

        if stage == 0:
            nc.sync.dma_start(out3[0:P, r0:r0 + band, :], oo[:].bitcast(F32))
        elif stage == 2:
            nc.sync.dma_start(
                out3[2 * P + r0:2 * P + r0 + band, :, :]
                .rearrange("c h w -> h c w"),
                oo[:].bitcast(F32))
        if stage != 2:
            # handoff: per row-slice PE transpose, 4 packed per PSUM bank
            for q0 in range(0, band, 4):
                pt = psum_t.tile([P, 4 * P], F32R, tag="pt", name="pt")
                for q in range(4):
                    nc.tensor.transpose(pt[:, q * P:(q + 1) * P],
                                        oo[:, q0 + q, :], identr[:])
                if stage == 0:
                    # in2_full[w, c, h]: dst iterates (c, h-in-pack);
                    # pack holds (q=h-slice, c) -> permute source
                    epi.copy(handoff_full[:, :, r0 + q0:r0 + q0 + 4],
                             pt[:].rearrange("p (q c) -> p c q", c=P))
                else:
                    # in3_full[h, c, w]: dst iterates (c-in-pack, w);
                    # pack holds (q=c-slice, w) -> orders already match
                    epi.copy(handoff_full[:, r0 + q0:r0 + q0 + 4, :], pt[:])


def _build_nc():
    nc = bacc.Bacc(None, target_bir_lowering=False)
    x_in = nc.dram_tensor("x", [P, S, S], F32, kind="ExternalInput")
    diags_in = nc.dram_tensor("diags", [P, N_MAT * P], F32, kind="ExternalInput")
    biases_in = nc.dram_tensor("biases", [P, 12], F32, kind="ExternalInput")
    out3 = nc.dram_tensor("out3", [3 * P, S, S], F32, kind="ExternalOutput")

    with tile.TileContext(nc) as tc:
        with tc.tile_pool(name="consts", bufs=1) as cpool, \
             tc.tile_pool(name="full2", bufs=1) as p_in2, \
             tc.tile_pool(name="psum", bufs=6, space="PSUM") as psum_pool, \
             tc.tile_pool(name="psum_t", bufs=2, space="PSUM") as psum_t:

            diags = cpool.tile([P, N_MAT * P], F32R, tag="diags")
            biases = cpool.tile([P, 12], F32, tag="biases")
            identr = cpool.tile([P, P], F32R, tag="identr")
            nc.sync.dma_start(diags[:], diags_in[:].bitcast(F32R))
            nc.sync.dma_start(biases[:], biases_in[:])
            nc.sync.dma_start(identr[:],
                              diags_in[:, IDENT * P:(IDENT + 1) * P].bitcast(F32R))

            in2_full = p_in2.tile([P, S, S], F32R, tag="in2")   # [w, c, h]
            epi = _Epi(nc)

            # Phase 1: stage 1 (partitions=C, rows=H, cols=W)
            with tc.tile_pool(name="s1work", bufs=1) as pw:
                _stage(nc, tc, epi, psum_pool, psum_t, diags, biases, identr,
                       0, pw, 16, None, x_in, out3, in2_full)

            # full3 pool opens only now: pools reserve SBUF from their open
            # point, and phase 1 needs the space for 32-row bands.
            with tc.tile_pool(name="full3", bufs=1) as p_in3:
                # Phase 2: stage 2 (partitions=W, rows=C, cols=H), 16-row bands
                in3_full = p_in3.tile([P, S, S], F32R, tag="in3")   # [h, c, w]
                with tc.tile_pool(name="s2work", bufs=1) as pw:
                    _stage(nc, tc, epi, psum_pool, psum_t, diags, biases,
                           identr, 1, pw, 16, in2_full, None, out3, in3_full)

                # block2 = in3_full[h, c, w] -> out3[P + c, h, w]
                nc.sync.dma_start(
                    out3[P:2 * P, :, :].rearrange("c h w -> h c w"),
                    in3_full[:].bitcast(F32))

                # Phase 3: stage 3 (partitions=H, rows=C, cols=W), 16-row bands
                with tc.tile_pool(name="s3work", bufs=1) as pw:
                    _stage(nc, tc, epi, psum_pool, psum_t, diags, biases,
                           identr, 2, pw, 16, in3_full, None, out3, None)
    nc.finalize()
    return nc


def _prep_weights(inputs):
    """Build diag matrices [P, N_MAT*P] and biases [P, 12] (BN folded)."""
    diags = np.zeros((P, N_MAT * P), np.float32)
    biases = np.zeros((P, 12), np.float32)
    idx = np.arange(P)
    for s, pfx in enumerate(("hw", "ch", "cw")):
        ws = [np.asarray(inputs[f"{pfx}_w{j}"]).reshape(P, 3) for j in (1, 2, 3, 4)]
        bs = [np.asarray(inputs[f"{pfx}_b{j}"]).astype(np.float32) for j in (1, 2, 3, 4)]
        g = np.asarray(inputs[f"{pfx}_g"])
        bt = np.asarray(inputs[f"{pfx}_bt"])
        m = np.asarray(inputs[f"{pfx}_m"])
        v = np.asarray(inputs[f"{pfx}_v"])
        scale = (g / np.sqrt(v + EPS)).astype(np.float32)
        for k in range(4):
            w = ws[k].astype(np.float32)
            if k == 3:
                w = w * scale[:, None]
                biases[:, s * 4 + k] = (bs[3] - m) * scale + bt
            else:
                biases[:, s * 4 + k] = bs[k]
            for t in range(3):
                j = s * 12 + k * 3 + t
                diags[idx, j * P + idx] = w[:, t]
    diags[idx, IDENT * P + idx] = 1.0
    return diags, biases


_NC_CACHE = None
TRACE = False            # set True (e.g. from test.py) to capture an NTFF profile
LAST_RESULT = None       # BassKernelResults of the most recent kernel() call


def _get_nc():
    global _NC_CACHE
    if _NC_CACHE is None:
        _NC_CACHE = _build_nc()
    return _NC_CACHE


def kernel(**inputs):
    global LAST_RESULT
    x = np.asarray(inputs["x"], np.float32)          # [8, 128, 128, 128]
    B = x.shape[0]
    diags, biases = _prep_weights(inputs)
    nc = _get_nc()
    in_maps = [{"x": np.ascontiguousarray(x[b]), "diags": diags,
                "biases": biases} for b in range(B)]
    res = run_bass_kernel_spmd(nc, in_maps, core_ids=list(range(B)),
                               trace=TRACE)
    LAST_RESULT = res
    out = np.empty((B, 4 * P, S, S), np.float32)
    out[:, :P] = x
    for b in range(B):
        out[b, P:] = res.results[b]["out3"]
    return out
